# revision 1
# baseline (speedup 1.0000x reference)
"""Causal self-attention (B=4, T=2048, C=768, H=12) on 8 Trainium2 cores.

Sharding (Megatron-style hybrid): core c handles batch b = c//2 and head-group
g = c%2 (6 heads, 384 channels). Q/K/V weights are column-split per group, the
output projection row-split; each core emits a partial y that the host sums
over the two groups of a batch (the "all-reduce" happens on the host since we
return full outputs anyway).

Per-core kernel (all activations kept transposed so no on-chip transposes are
ever needed; contraction dim always lands on SBUF partitions):
  Q^T = Wq_g^T x^T   [384, T]      K^T likewise      V = x Wv_g  [T, 384]
  per head pair (rows 0:64 head A / 64:128 head B of a 128-row feature chunk):
    S^T tile = (K^T tile)^T-contract: matmul(lhsT=K^T[d,kt], rhs=Q^T[d,qr])
    P~ = exp(S^T/8) (ScalarE, bf16 out), causal-masked by mask-tile multiply
    O~^T/l = matmul(lhsT=[V|1], rhs=P~) accumulated over k tiles  (ones row
             makes PSUM row 64 the softmax denominator l)
    O^T = O~^T * broadcast(1/l)   (reciprocal_approx_fast + DMA broadcast)
  y = O Wp_g  [T, 768] fp32 -> HBM

Inputs are pre-transposed/cast to bf16 on the host (data layout prep only; all
matmuls, softmax and normalization run on device).
"""

import numpy as np
import ml_dtypes

import concourse.bass as bass
import concourse.tile as tile
import concourse.mybir as mybir
from concourse import bacc
from concourse.bass_utils import run_bass_kernel_spmd

F32 = mybir.dt.float32
BF16 = mybir.dt.bfloat16
BF = ml_dtypes.bfloat16

B, T, C, H, D = 4, 2048, 768, 12, 64
G = 2                    # head groups (tensor-parallel degree)
HG = H // G              # heads per group = 6
CG = C // G              # channels per group = 384
HP = HG // 2             # head pairs per group = 3
KC = C // 128            # contraction chunks over C = 6
QR = T // 512            # 512-wide q ranges = 4
TT = T // 128            # 128-wide token tiles = 16
NCORES = 8
SCALE = 1.0 / np.sqrt(D)

_nc_cache = {}


def _build_nc():
    nc = bacc.Bacc(None, target_bir_lowering=False, debug=False,
                   num_devices=NCORES, name="csa")

    xt_d = nc.dram_tensor("xt", [C, T], BF16, kind="ExternalInput")
    wq_d = nc.dram_tensor("wq", [C, CG], BF16, kind="ExternalInput")
    wk_d = nc.dram_tensor("wk", [C, CG], BF16, kind="ExternalInput")
    wv_d = nc.dram_tensor("wv", [C, CG], BF16, kind="ExternalInput")
    wp_d = nc.dram_tensor("wp", [CG, C], BF16, kind="ExternalInput")
    mk_d = nc.dram_tensor("masks", [4, 128, 512], BF16, kind="ExternalInput")
    bqk_d = nc.dram_tensor("bqk", [128, 2 * HP], F32, kind="ExternalInput")
    bv_d = nc.dram_tensor("bv_bc", [128, CG], F32, kind="ExternalInput")
    y_d = nc.dram_tensor("y", [T, C], F32, kind="ExternalOutput")

    with tile.TileContext(nc) as tc:
        with (
            tc.tile_pool(name="persist", bufs=1) as pers,
            tc.tile_pool(name="work", bufs=3) as work,
            tc.tile_pool(name="norm", bufs=4) as nrm,
        ):
            # ---- persistent SBUF tensors ----
            xt_t = pers.tile([128, KC, T], BF16)
            wq_t = pers.tile([128, KC, CG], BF16)
            wk_t = pers.tile([128, KC, CG], BF16)
            wv_t = pers.tile([128, KC, CG], BF16)
            wp_t = pers.tile([128, HP, C], BF16)
            mk_t = pers.tile([128, 4, 512], BF16)
            bqk_t = pers.tile([128, 2 * HP], F32)
            bv_t = pers.tile([128, CG], F32)
            qt_t = pers.tile([128, HP, T], BF16)
            kt_t = pers.tile([128, HP, T], BF16)
            v1_t = pers.tile([128, TT, HG, D + 1], BF16)
            on_t = pers.tile([128, HP, T], BF16)

            nc.gpsimd.dma_start(xt_t[:], xt_d[:].rearrange("(kc p) t -> p kc t", p=128))
            nc.gpsimd.dma_start(wq_t[:], wq_d[:].rearrange("(kc p) f -> p kc f", p=128))
            nc.gpsimd.dma_start(wk_t[:], wk_d[:].rearrange("(kc p) f -> p kc f", p=128))
            nc.gpsimd.dma_start(wv_t[:], wv_d[:].rearrange("(kc p) f -> p kc f", p=128))
            nc.gpsimd.dma_start(wp_t[:], wp_d[:].rearrange("(hp p) f -> p hp f", p=128))
            nc.gpsimd.dma_start(mk_t[:], mk_d[:].rearrange("j p q -> p j q"))
            nc.gpsimd.dma_start(bqk_t[:], bqk_d[:])
            nc.gpsimd.dma_start(bv_t[:], bv_d[:])
            nc.vector.memset(v1_t[:, :, :, D:D + 1], 1.0)

            # ---- phase 1: projections ----
            with tc.tile_pool(name="pj", bufs=3, space="PSUM") as pj:
                for fc in range(HP):
                    for qr in range(QR):
                        q_ps = pj.tile([128, 512], F32, tag="qk")
                        for kc in range(KC):
                            nc.tensor.matmul(
                                q_ps[:], wq_t[:, kc, fc * 128:(fc + 1) * 128],
                                xt_t[:, kc, qr * 512:(qr + 1) * 512],
                                start=(kc == 0), stop=(kc == KC - 1))
                        nc.vector.tensor_scalar_add(
                            qt_t[:, fc, qr * 512:(qr + 1) * 512], q_ps[:],
                            bqk_t[:, fc:fc + 1])
                        k_ps = pj.tile([128, 512], F32, tag="qk")
                        for kc in range(KC):
                            nc.tensor.matmul(
                                k_ps[:], wk_t[:, kc, fc * 128:(fc + 1) * 128],
                                xt_t[:, kc, qr * 512:(qr + 1) * 512],
                                start=(kc == 0), stop=(kc == KC - 1))
                        nc.vector.tensor_scalar_add(
                            kt_t[:, fc, qr * 512:(qr + 1) * 512], k_ps[:],
                            bqk_t[:, HP + fc:HP + fc + 1])
                for tt in range(TT):
                    v_ps = pj.tile([128, CG], F32, tag="v")
                    for kc in range(KC):
                        nc.tensor.matmul(
                            v_ps[:], xt_t[:, kc, tt * 128:(tt + 1) * 128],
                            wv_t[:, kc, :],
                            start=(kc == 0), stop=(kc == KC - 1))
                    nc.vector.tensor_tensor(
                        v1_t[:, tt, :, 0:D],
                        v_ps[:].rearrange("p (h d) -> p h d", h=HG),
                        bv_t[:].rearrange("p (h d) -> p h d", h=HG),
                        mybir.AluOpType.add)

            # ---- phase 2: attention (transposed flash, no running max) ----
            with (
                tc.tile_pool(name="ps", bufs=2, space="PSUM") as ps,
                tc.tile_pool(name="po", bufs=2, space="PSUM") as po,
            ):
                for qr in range(QR):
                    nki = 4 * qr + 4
                    for hp in range(HP):
                        o_ps = po.tile([128, 1024], F32, tag="o")
                        for ki in range(nki):
                            s_ps = ps.tile([128, 1024], F32, tag="s")
                            nc.tensor.matmul(
                                s_ps[:, 0:512],
                                kt_t[0:64, hp, ki * 128:(ki + 1) * 128],
                                qt_t[0:64, hp, qr * 512:(qr + 1) * 512],
                                start=True, stop=True)
                            nc.tensor.matmul(
                                s_ps[:, 512:1024],
                                kt_t[64:128, hp, ki * 128:(ki + 1) * 128],
                                qt_t[64:128, hp, qr * 512:(qr + 1) * 512],
                                start=True, stop=True)
                            p_t = work.tile([128, 1024], BF16, tag="p")
                            nc.scalar.activation(
                                p_t[:], s_ps[:],
                                mybir.ActivationFunctionType.Exp, scale=SCALE)
                            j = ki - 4 * qr
                            if j >= 0:
                                p3 = p_t[:].rearrange("p (two q) -> p two q", two=2)
                                m3 = mk_t[:, j:j + 1, :].to_broadcast((128, 2, 512))
                                nc.vector.tensor_tensor(
                                    p3, p3, m3, mybir.AluOpType.mult)
                            nc.tensor.matmul(
                                o_ps[0:D + 1, 0:512],
                                v1_t[:, ki, 2 * hp, :], p_t[:, 0:512],
                                start=(ki == 0), stop=(ki == nki - 1))
                            nc.tensor.matmul(
                                o_ps[0:D + 1, 512:1024],
                                v1_t[:, ki, 2 * hp + 1, :], p_t[:, 512:1024],
                                start=(ki == 0), stop=(ki == nki - 1))
                        # softmax denominators: recip of PSUM row 64, bcast, mul
                        l_row = nrm.tile([1, 1024], F32, tag="l")
                        nc.vector.tensor_copy(l_row[:], o_ps[D:D + 1, :])
                        r_row = nrm.tile([1, 1024], F32, tag="r")
                        nc.vector.reciprocal_approx_fast(out=r_row[:], in_=l_row[:])
                        r_bc = nrm.tile([64, 1024], F32, tag="rb")
                        rap = r_row[:]
                        nc.gpsimd.dma_start(r_bc[:], bass.AP(
                            tensor=rap.tensor, offset=rap.offset,
                            ap=[list(rap.ap[0]), [0, 64]] + [list(a) for a in rap.ap[1:]]))
                        nc.vector.tensor_tensor(
                            on_t[0:64, hp, qr * 512:(qr + 1) * 512],
                            o_ps[0:64, 0:512], r_bc[:, 0:512],
                            mybir.AluOpType.mult)
                        ob_t = nrm.tile([64, 512], BF16, tag="ob")
                        nc.vector.tensor_tensor(
                            ob_t[:], o_ps[0:64, 512:1024], r_bc[:, 512:1024],
                            mybir.AluOpType.mult)
                        nc.gpsimd.dma_start(
                            on_t[64:128, hp, qr * 512:(qr + 1) * 512], ob_t[:])

            # ---- phase 3: output projection ----
            with tc.tile_pool(name="py", bufs=2, space="PSUM") as py:
                for tt in range(TT):
                    ya_ps = py.tile([128, 512], F32, tag="ya")
                    yb_ps = py.tile([128, 256], F32, tag="yb")
                    for hp in range(HP):
                        lhsT = on_t[:, hp, tt * 128:(tt + 1) * 128]
                        nc.tensor.matmul(ya_ps[:], lhsT, wp_t[:, hp, 0:512],
                                         start=(hp == 0), stop=(hp == HP - 1))
                        nc.tensor.matmul(yb_ps[:], lhsT, wp_t[:, hp, 512:768],
                                         start=(hp == 0), stop=(hp == HP - 1))
                    y_sb = work.tile([128, C], F32, tag="y")
                    nc.vector.tensor_copy(y_sb[:, 0:512], ya_ps[:])
                    nc.vector.tensor_copy(y_sb[:, 512:768], yb_ps[:])
                    nc.gpsimd.dma_start(y_d[tt * 128:(tt + 1) * 128, :], y_sb[:])

    nc.finalize()
    return nc


def _causal_masks():
    q = np.arange(512)[None, :]
    k = np.arange(128)[:, None]
    return np.stack([(q >= k + j * 128) for j in range(4)]).astype(BF)


def kernel(x, Wq, bq, Wk, bk, Wv, bv, Wp, bp):
    x, Wq, bq, Wk, bk, Wv, bv, Wp, bp = (
        np.asarray(a, dtype=np.float32)
        for a in (x, Wq, bq, Wk, bk, Wv, bv, Wp, bp))

    if "nc" not in _nc_cache:
        _nc_cache["nc"] = _build_nc()
    nc = _nc_cache["nc"]

    masks = _causal_masks()
    in_maps = []
    for c in range(NCORES):
        b, g = c // 2, c % 2
        sl = slice(g * CG, (g + 1) * CG)
        bqk = np.concatenate([bq[sl].reshape(HP, 128).T,
                              bk[sl].reshape(HP, 128).T], axis=1)
        in_maps.append({
            "xt": np.ascontiguousarray(x[b].T).astype(BF),
            "wq": np.ascontiguousarray(Wq[:, sl]).astype(BF),
            "wk": np.ascontiguousarray(Wk[:, sl]).astype(BF),
            "wv": np.ascontiguousarray(Wv[:, sl]).astype(BF),
            "wp": np.ascontiguousarray(Wp[sl, :]).astype(BF),
            "masks": masks,
            "bqk": np.ascontiguousarray(bqk).astype(np.float32),
            "bv_bc": np.tile(bv[sl][None, :], (128, 1)).astype(np.float32),
        })

    res = run_bass_kernel_spmd(nc, in_maps, core_ids=list(range(NCORES)))
    out = np.empty((B, T, C), np.float32)
    for b in range(B):
        out[b] = res.results[2 * b]["y"] + res.results[2 * b + 1]["y"] + bp
    return out


# revision 11
# speedup vs baseline: 6552.5860x; 6552.5860x over previous
"""Causal self-attention (B=4, T=2048, C=768, H=12) on 8 Trainium2 cores.

Sharding (Megatron-style hybrid): core c handles batch b = c//2 and head-group
g = c%2 (6 heads, 384 channels). Q/K/V weights are column-split per group, the
output projection row-split; each core emits a partial y that the host sums
over the two groups of a batch (the "all-reduce" happens on the host since we
return full outputs anyway).

Per-core kernel (all activations kept transposed so no on-chip transposes are
ever needed; contraction dim always lands on SBUF partitions):
  Q^T = Wq_g^T x^T   [384, T]      K^T likewise      V = x Wv_g  [T, 384]
  per head pair (rows 0:64 head A / 64:128 head B of a 128-row feature chunk):
    S^T tile = (K^T tile)^T-contract: matmul(lhsT=K^T[d,kt], rhs=Q^T[d,qr])
    P~ = exp(S^T/8) (ScalarE, bf16 out), causal-masked by mask-tile multiply
    O~^T/l = matmul(lhsT=[V|1], rhs=P~) accumulated over k tiles  (ones row
             makes PSUM row 64 the softmax denominator l)
    O^T = O~^T * broadcast(1/l)   (reciprocal_approx_fast + DMA broadcast)
  y = O Wp_g  [T, 768] fp32 -> HBM

Inputs are pre-transposed/cast to bf16 on the host (data layout prep only; all
matmuls, softmax and normalization run on device).
"""

import numpy as np
import ml_dtypes

import concourse.bass as bass
import concourse.tile as tile
import concourse.mybir as mybir
from concourse import bacc
from concourse.bass_utils import run_bass_kernel_spmd

F32 = mybir.dt.float32
BF16 = mybir.dt.bfloat16
BF = ml_dtypes.bfloat16

B, T, C, H, D = 4, 2048, 768, 12, 64
G = 2                    # head groups (tensor-parallel degree)
HG = H // G              # heads per group = 6
CG = C // G              # channels per group = 384
HP = HG // 2             # head pairs per group = 3
KC = C // 128            # contraction chunks over C = 6
QR = T // 512            # 512-wide q ranges = 4
TT = T // 128            # 128-wide token tiles = 16
NCORES = 8
SCALE = 1.0 / np.sqrt(D)

_nc_cache = {}


def _build_nc(reps=1, loop=False):
    nc = bacc.Bacc(None, target_bir_lowering=False, debug=False,
                   num_devices=NCORES, name="csa")
    if loop:
        ni_d = nc.dram_tensor("niter", [1, 1], mybir.dt.int32, kind="ExternalInput")

    xt_d = nc.dram_tensor("xt", [C, T], BF16, kind="ExternalInput")
    wq_d = nc.dram_tensor("wq", [C, CG], BF16, kind="ExternalInput")
    wk_d = nc.dram_tensor("wk", [C, CG], BF16, kind="ExternalInput")
    wv_d = nc.dram_tensor("wv", [C, CG], BF16, kind="ExternalInput")
    wp_d = nc.dram_tensor("wp", [CG, C], BF16, kind="ExternalInput")
    mk_d = nc.dram_tensor("masks", [128, 128], BF16, kind="ExternalInput")
    bqk_d = nc.dram_tensor("bqk", [128, 2 * HP], F32, kind="ExternalInput")
    bv_d = nc.dram_tensor("bv_bc", [128, CG], F32, kind="ExternalInput")
    y_d = nc.dram_tensor("y", [T, C], F32, kind="ExternalOutput")

    with tile.TileContext(nc) as tc:
        with (
            tc.tile_pool(name="persist", bufs=1) as pers,
            tc.tile_pool(name="work", bufs=6) as work,
            tc.tile_pool(name="norm", bufs=6) as nrm,
        ):
            # ---- persistent SBUF tensors ----
            xt_t = pers.tile([128, KC, T], BF16)
            wq_t = pers.tile([128, KC, CG], BF16)
            wk_t = pers.tile([128, KC, CG], BF16)
            wv_t = pers.tile([128, KC, CG], BF16)
            wp_t = pers.tile([128, HP, C], BF16)
            mk_t = pers.tile([128, 128], BF16)
            bqk_t = pers.tile([128, 2 * HP], F32)
            bv_t = pers.tile([128, CG], F32)
            qt_t = pers.tile([128, HP, T], BF16)
            kt_t = pers.tile([128, HP, T], BF16)
            v1_t = pers.tile([128, TT, HG, D + 1], BF16)
            on_t = pers.tile([128, HP, T], BF16)

            xt_r = xt_d[:].rearrange("(kc p) t -> p kc t", p=128)
            for kc in range(KC):
                nc.gpsimd.dma_start(xt_t[:, kc, :], xt_r[:, kc, :])
            nc.gpsimd.dma_start(wq_t[:], wq_d[:].rearrange("(kc p) f -> p kc f", p=128))
            nc.gpsimd.dma_start(wk_t[:], wk_d[:].rearrange("(kc p) f -> p kc f", p=128))
            nc.gpsimd.dma_start(wv_t[:], wv_d[:].rearrange("(kc p) f -> p kc f", p=128))
            nc.gpsimd.dma_start(wp_t[:], wp_d[:].rearrange("(hp p) f -> p hp f", p=128))
            nc.gpsimd.dma_start(mk_t[:], mk_d[:])
            nc.gpsimd.dma_start(bqk_t[:], bqk_d[:])
            nc.gpsimd.dma_start(bv_t[:], bv_d[:])
            nc.vector.memset(v1_t[:, :, :, D:D + 1], 1.0)

            if loop:
                ni_t = pers.tile([1, 1], mybir.dt.int32)
                nc.gpsimd.dma_start(ni_t[:], ni_d[:])
                ni_reg = nc.values_load(ni_t[0:1, 0:1].to_broadcast((1, 1)))
                loop_cm = tc.For_i(0, ni_reg, 1)
                loop_cm.__enter__()

            for _rep in range(reps):
                # ---- phase 1: projections ----
                with tc.tile_pool(name="pj", bufs=3, space="PSUM") as pj:
                    for fc in range(HP):
                        for qr in range(QR):
                            q_ps = pj.tile([128, 512], F32, tag="qk")
                            for kc in range(KC):
                                nc.tensor.matmul(
                                    q_ps[:], wq_t[:, kc, fc * 128:(fc + 1) * 128],
                                    xt_t[:, kc, qr * 512:(qr + 1) * 512],
                                    start=(kc == 0), stop=(kc == KC - 1))
                            nc.scalar.activation(
                                qt_t[:, fc, qr * 512:(qr + 1) * 512], q_ps[:],
                                mybir.ActivationFunctionType.Identity,
                                bias=bqk_t[:, fc:fc + 1])
                            k_ps = pj.tile([128, 512], F32, tag="qk")
                            for kc in range(KC):
                                nc.tensor.matmul(
                                    k_ps[:], wk_t[:, kc, fc * 128:(fc + 1) * 128],
                                    xt_t[:, kc, qr * 512:(qr + 1) * 512],
                                    start=(kc == 0), stop=(kc == KC - 1))
                            nc.scalar.activation(
                                kt_t[:, fc, qr * 512:(qr + 1) * 512], k_ps[:],
                                mybir.ActivationFunctionType.Identity,
                                bias=bqk_t[:, HP + fc:HP + fc + 1])
                    for tt in range(TT):
                        v_ps = pj.tile([128, CG], F32, tag="v")
                        for kc in range(KC):
                            nc.tensor.matmul(
                                v_ps[:], xt_t[:, kc, tt * 128:(tt + 1) * 128],
                                wv_t[:, kc, :],
                                start=(kc == 0), stop=(kc == KC - 1))
                        nc.vector.tensor_tensor(
                            v1_t[:, tt, :, 0:D],
                            v_ps[:].rearrange("p (h d) -> p h d", h=HG),
                            bv_t[:].rearrange("p (h d) -> p h d", h=HG),
                            mybir.AluOpType.add)

                # ---- phase 2: attention (transposed flash, no running max) ----
                with (
                    tc.tile_pool(name="ps", bufs=2, space="PSUM") as ps,
                    tc.tile_pool(name="po", bufs=2, space="PSUM") as po,
                ):
                    for qr in range(QR):
                        nki = 4 * qr + 4
                        for hp in range(HP):
                            o_ps = po.tile([128, 1024], F32, tag="o")
                            for ki in range(nki):
                                s_ps = ps.tile([128, 1024], F32, tag="s")
                                nc.tensor.matmul(
                                    s_ps[:, 0:512],
                                    kt_t[0:64, hp, ki * 128:(ki + 1) * 128],
                                    qt_t[0:64, hp, qr * 512:(qr + 1) * 512],
                                    start=True, stop=True)
                                nc.tensor.matmul(
                                    s_ps[:, 512:1024],
                                    kt_t[64:128, hp, ki * 128:(ki + 1) * 128],
                                    qt_t[64:128, hp, qr * 512:(qr + 1) * 512],
                                    start=True, stop=True)
                                p_t = work.tile([128, 1024], BF16, tag="p")
                                j = ki - 4 * qr
                                p3 = p_t[:].rearrange("p (two q) -> p two q", two=2)
                                s3 = s_ps[:].rearrange("p (two q) -> p two q", two=2)
                                if j <= 0:
                                    nc.scalar.activation(
                                        p_t[:], s_ps[:],
                                        mybir.ActivationFunctionType.Exp, scale=SCALE)
                                else:
                                    # diag block: exp only cols >= j*128; the
                                    # skipped cols are memset to 0 below
                                    nc.scalar.activation(
                                        p3[:, :, j * 128:], s3[:, :, j * 128:],
                                        mybir.ActivationFunctionType.Exp, scale=SCALE)
                                if j > 0:
                                    nc.vector.memset(p3[:, :, 0:128 * j], 0.0)
                                if j >= 0:
                                    tri = p3[:, :, 128 * j:128 * (j + 1)]
                                    m3 = mk_t[:, None, :].to_broadcast((128, 2, 128))
                                    nc.vector.tensor_tensor(
                                        tri, tri, m3, mybir.AluOpType.mult)
                                nc.tensor.matmul(
                                    o_ps[0:D + 1, 0:512],
                                    v1_t[:, ki, 2 * hp, :], p_t[:, 0:512],
                                    start=(ki == 0), stop=(ki == nki - 1))
                                nc.tensor.matmul(
                                    o_ps[0:D + 1, 512:1024],
                                    v1_t[:, ki, 2 * hp + 1, :], p_t[:, 512:1024],
                                    start=(ki == 0), stop=(ki == nki - 1))
                            # softmax denominators: recip of PSUM row 64, bcast, mul
                            l_row = nrm.tile([1, 1024], F32, tag="l")
                            nc.vector.tensor_copy(l_row[:], o_ps[D:D + 1, :])
                            r_row = nrm.tile([1, 1024], F32, tag="r")
                            nc.vector.reciprocal_approx_fast(out=r_row[:], in_=l_row[:])
                            r_bc = nrm.tile([64, 1024], F32, tag="rb")
                            rap = r_row[:]
                            nc.sync.dma_start(r_bc[:], bass.AP(
                                tensor=rap.tensor, offset=rap.offset,
                                ap=[list(rap.ap[0]), [0, 64]] + [list(a) for a in rap.ap[1:]]))
                            nc.vector.tensor_tensor(
                                on_t[0:64, hp, qr * 512:(qr + 1) * 512],
                                o_ps[0:64, 0:512], r_bc[:, 0:512],
                                mybir.AluOpType.mult)
                            ob_t = nrm.tile([64, 512], BF16, tag="ob")
                            nc.vector.tensor_tensor(
                                ob_t[:], o_ps[0:64, 512:1024], r_bc[:, 512:1024],
                                mybir.AluOpType.mult)
                            nc.sync.dma_start(
                                on_t[64:128, hp, qr * 512:(qr + 1) * 512], ob_t[:])

                # ---- phase 3: output projection ----
                with tc.tile_pool(name="py", bufs=2, space="PSUM") as py:
                    for tt in range(TT):
                        ya_ps = py.tile([128, 512], F32, tag="ya")
                        yb_ps = py.tile([128, 256], F32, tag="yb")
                        for hp in range(HP):
                            lhsT = on_t[:, hp, tt * 128:(tt + 1) * 128]
                            nc.tensor.matmul(ya_ps[:], lhsT, wp_t[:, hp, 0:512],
                                             start=(hp == 0), stop=(hp == HP - 1))
                            nc.tensor.matmul(yb_ps[:], lhsT, wp_t[:, hp, 512:768],
                                             start=(hp == 0), stop=(hp == HP - 1))
                        y_sb = work.tile([128, C], F32, tag="y")
                        nc.vector.tensor_copy(y_sb[:, 0:512], ya_ps[:])
                        nc.vector.tensor_copy(y_sb[:, 512:768], yb_ps[:])
                        nc.gpsimd.dma_start(y_d[tt * 128:(tt + 1) * 128, :], y_sb[:])

            if loop:
                loop_cm.__exit__(None, None, None)

    nc.finalize()
    return nc


def _causal_masks():
    f = np.arange(128)[None, :]
    k = np.arange(128)[:, None]
    return (f >= k).astype(BF)


def kernel(x, Wq, bq, Wk, bk, Wv, bv, Wp, bp):
    x, Wq, bq, Wk, bk, Wv, bv, Wp, bp = (
        np.asarray(a, dtype=np.float32)
        for a in (x, Wq, bq, Wk, bk, Wv, bv, Wp, bp))

    if "nc" not in _nc_cache:
        _nc_cache["nc"] = _build_nc()
    nc = _nc_cache["nc"]

    masks = _causal_masks()
    in_maps = []
    for c in range(NCORES):
        b, g = c // 2, c % 2
        sl = slice(g * CG, (g + 1) * CG)
        bqk = np.concatenate([bq[sl].reshape(HP, 128).T,
                              bk[sl].reshape(HP, 128).T], axis=1)
        in_maps.append({
            "xt": np.ascontiguousarray(x[b].T).astype(BF),
            "wq": np.ascontiguousarray(Wq[:, sl]).astype(BF),
            "wk": np.ascontiguousarray(Wk[:, sl]).astype(BF),
            "wv": np.ascontiguousarray(Wv[:, sl]).astype(BF),
            "wp": np.ascontiguousarray(Wp[sl, :]).astype(BF),
            "masks": masks,
            "bqk": np.ascontiguousarray(bqk).astype(np.float32),
            "bv_bc": np.tile(bv[sl][None, :], (128, 1)).astype(np.float32),
        })

    res = run_bass_kernel_spmd(nc, in_maps, core_ids=list(range(NCORES)))
    out = np.empty((B, T, C), np.float32)
    for b in range(B):
        out[b] = res.results[2 * b]["y"] + res.results[2 * b + 1]["y"] + bp
    return out



# revision 12
# speedup vs baseline: 9475.3165x; 1.4460x over previous
"""Causal self-attention (B=4, T=2048, C=768, H=12) on 8 Trainium2 cores.

Sharding (Megatron-style hybrid): core c handles batch b = c//2 and head-group
g = c%2 (6 heads, 384 channels). Q/K/V weights are column-split per group, the
output projection row-split; each core emits a partial y that the host sums
over the two groups of a batch (the "all-reduce" happens on the host since we
return full outputs anyway).

Per-core kernel (all activations kept transposed so no on-chip transposes are
ever needed; contraction dim always lands on SBUF partitions):
  Q^T = Wq_g^T x^T   [384, T]      K^T likewise      V = x Wv_g  [T, 384]
  per head pair (rows 0:64 head A / 64:128 head B of a 128-row feature chunk):
    S^T tile = (K^T tile)^T-contract: matmul(lhsT=K^T[d,kt], rhs=Q^T[d,qr])
    P~ = exp(S^T/8) (ScalarE, bf16 out), causal-masked by mask-tile multiply
    O~^T/l = matmul(lhsT=[V|1], rhs=P~) accumulated over k tiles  (ones row
             makes PSUM row 64 the softmax denominator l)
    O^T = O~^T * broadcast(1/l)   (reciprocal_approx_fast + DMA broadcast)
  y = O Wp_g  [T, 768] fp32 -> HBM

Inputs are pre-transposed/cast to bf16 on the host (data layout prep only; all
matmuls, softmax and normalization run on device).
"""

import numpy as np
import ml_dtypes

import concourse.bass as bass
import concourse.tile as tile
from concourse import library_config
import concourse.mybir as mybir
from concourse import bacc
from concourse.bass_utils import run_bass_kernel_spmd

F32 = mybir.dt.float32
BF16 = mybir.dt.bfloat16
BF = ml_dtypes.bfloat16

B, T, C, H, D = 4, 2048, 768, 12, 64
G = 2                    # head groups (tensor-parallel degree)
HG = H // G              # heads per group = 6
CG = C // G              # channels per group = 384
HP = HG // 2             # head pairs per group = 3
KC = C // 128            # contraction chunks over C = 6
QR = T // 512            # 512-wide q ranges = 4
TT = T // 128            # 128-wide token tiles = 16
NCORES = 8
SCALE = 1.0 / np.sqrt(D)

_nc_cache = {}


def _build_nc(reps=1, loop=False):
    nc = bacc.Bacc(None, target_bir_lowering=False, debug=False,
                   num_devices=NCORES, name="csa")
    if loop:
        ni_d = nc.dram_tensor("niter", [1, 1], mybir.dt.int32, kind="ExternalInput")

    xt_d = nc.dram_tensor("xt", [C, T], BF16, kind="ExternalInput")
    wq_d = nc.dram_tensor("wq", [C, CG], BF16, kind="ExternalInput")
    wk_d = nc.dram_tensor("wk", [C, CG], BF16, kind="ExternalInput")
    wv_d = nc.dram_tensor("wv", [C, CG], BF16, kind="ExternalInput")
    wp_d = nc.dram_tensor("wp", [CG, C], BF16, kind="ExternalInput")
    mk_d = nc.dram_tensor("masks", [128, 128], BF16, kind="ExternalInput")
    bqk_d = nc.dram_tensor("bqk", [128, 2 * HP], F32, kind="ExternalInput")
    bv_d = nc.dram_tensor("bv_bc", [128, CG], F32, kind="ExternalInput")
    y_d = nc.dram_tensor("y", [T, C], F32, kind="ExternalOutput")

    with tile.TileContext(nc) as tc:
        with (
            tc.tile_pool(name="persist", bufs=1) as pers,
            tc.tile_pool(name="work", bufs=6) as work,
            tc.tile_pool(name="norm", bufs=6) as nrm,
        ):
            # ---- persistent SBUF tensors ----
            xt_t = pers.tile([128, KC, T], BF16)
            wq_t = pers.tile([128, KC, CG], BF16)
            wk_t = pers.tile([128, KC, CG], BF16)
            wv_t = pers.tile([128, KC, CG], BF16)
            wp_t = pers.tile([128, HP, C], BF16)
            mk_t = pers.tile([128, 128], BF16)
            bqk_t = pers.tile([128, 2 * HP], F32)
            bv_t = pers.tile([128, CG], F32)
            qt_t = pers.tile([128, HP, T], BF16)
            kt_t = pers.tile([128, HP, T], BF16)
            v1_t = pers.tile([128, TT, HG, D + 1], BF16)
            on_t = pers.tile([128, HP, T], BF16)

            xt_r = xt_d[:].rearrange("(kc p) t -> p kc t", p=128)
            for kc in range(KC):
                nc.gpsimd.dma_start(xt_t[:, kc, :], xt_r[:, kc, :])
            nc.gpsimd.dma_start(wq_t[:], wq_d[:].rearrange("(kc p) f -> p kc f", p=128))
            nc.gpsimd.dma_start(wk_t[:], wk_d[:].rearrange("(kc p) f -> p kc f", p=128))
            nc.gpsimd.dma_start(wv_t[:], wv_d[:].rearrange("(kc p) f -> p kc f", p=128))
            nc.gpsimd.dma_start(wp_t[:], wp_d[:].rearrange("(hp p) f -> p hp f", p=128))
            nc.gpsimd.dma_start(mk_t[:], mk_d[:])
            nc.gpsimd.dma_start(bqk_t[:], bqk_d[:])
            nc.gpsimd.dma_start(bv_t[:], bv_d[:])
            nc.vector.memset(v1_t[:, :, :, D:D + 1], 1.0)
            nc.gpsimd.load_library(library_config.attn)

            if loop:
                ni_t = pers.tile([1, 1], mybir.dt.int32)
                nc.gpsimd.dma_start(ni_t[:], ni_d[:])
                ni_reg = nc.values_load(ni_t[0:1, 0:1].to_broadcast((1, 1)))
                loop_cm = tc.For_i(0, ni_reg, 1)
                loop_cm.__enter__()

            for _rep in range(reps):
                # ---- phase 1: projections ----
                with tc.tile_pool(name="pj", bufs=3, space="PSUM") as pj:
                    for fc in range(HP):
                        for qr in range(QR):
                            q_ps = pj.tile([128, 512], F32, tag="qk")
                            for kc in range(KC):
                                nc.tensor.matmul(
                                    q_ps[:], wq_t[:, kc, fc * 128:(fc + 1) * 128],
                                    xt_t[:, kc, qr * 512:(qr + 1) * 512],
                                    start=(kc == 0), stop=(kc == KC - 1))
                            nc.scalar.activation(
                                qt_t[:, fc, qr * 512:(qr + 1) * 512], q_ps[:],
                                mybir.ActivationFunctionType.Identity,
                                bias=bqk_t[:, fc:fc + 1])
                            k_ps = pj.tile([128, 512], F32, tag="qk")
                            for kc in range(KC):
                                nc.tensor.matmul(
                                    k_ps[:], wk_t[:, kc, fc * 128:(fc + 1) * 128],
                                    xt_t[:, kc, qr * 512:(qr + 1) * 512],
                                    start=(kc == 0), stop=(kc == KC - 1))
                            nc.scalar.activation(
                                kt_t[:, fc, qr * 512:(qr + 1) * 512], k_ps[:],
                                mybir.ActivationFunctionType.Identity,
                                bias=bqk_t[:, HP + fc:HP + fc + 1])
                    for tt in range(TT):
                        v_ps = pj.tile([128, CG], F32, tag="v")
                        for kc in range(KC):
                            nc.tensor.matmul(
                                v_ps[:], xt_t[:, kc, tt * 128:(tt + 1) * 128],
                                wv_t[:, kc, :],
                                start=(kc == 0), stop=(kc == KC - 1))
                        nc.vector.tensor_tensor(
                            v1_t[:, tt, :, 0:D],
                            v_ps[:].rearrange("p (h d) -> p h d", h=HG),
                            bv_t[:].rearrange("p (h d) -> p h d", h=HG),
                            mybir.AluOpType.add)

                # ---- phase 2: attention (transposed flash, no running max) ----
                with (
                    tc.tile_pool(name="ps", bufs=2, space="PSUM") as ps,
                    tc.tile_pool(name="po", bufs=2, space="PSUM") as po,
                ):
                    for qr in range(QR):
                        nki = 4 * qr + 4
                        for hp in range(HP):
                            o_ps = po.tile([128, 1024], F32, tag="o")
                            for ki in range(nki):
                                s_ps = ps.tile([128, 1024], F32, tag="s")
                                nc.tensor.matmul(
                                    s_ps[:, 0:512],
                                    kt_t[0:64, hp, ki * 128:(ki + 1) * 128],
                                    qt_t[0:64, hp, qr * 512:(qr + 1) * 512],
                                    start=True, stop=True)
                                nc.tensor.matmul(
                                    s_ps[:, 512:1024],
                                    kt_t[64:128, hp, ki * 128:(ki + 1) * 128],
                                    qt_t[64:128, hp, qr * 512:(qr + 1) * 512],
                                    start=True, stop=True)
                                p_t = work.tile([128, 1024], BF16, tag="p")
                                j = ki - 4 * qr
                                p3 = p_t[:].rearrange("p (two q) -> p two q", two=2)
                                s3 = s_ps[:].rearrange("p (two q) -> p two q", two=2)
                                if j <= 0:
                                    nc.scalar.activation(
                                        p_t[:], s_ps[:],
                                        mybir.ActivationFunctionType.Exp, scale=SCALE)
                                else:
                                    # diag block: exp only cols >= j*128; the
                                    # skipped cols are memset to 0 below
                                    nc.scalar.activation(
                                        p3[:, :, j * 128:], s3[:, :, j * 128:],
                                        mybir.ActivationFunctionType.Exp, scale=SCALE)
                                if j > 0:
                                    nc.vector.memset(p3[:, :, 0:128 * j], 0.0)
                                if j >= 0:
                                    tri = p3[:, :, 128 * j:128 * (j + 1)]
                                    m3 = mk_t[:, None, :].to_broadcast((128, 2, 128))
                                    nc.vector.tensor_tensor(
                                        tri, tri, m3, mybir.AluOpType.mult)
                                nc.tensor.matmul(
                                    o_ps[0:D + 1, 0:512],
                                    v1_t[:, ki, 2 * hp, :], p_t[:, 0:512],
                                    start=(ki == 0), stop=(ki == nki - 1))
                                nc.tensor.matmul(
                                    o_ps[0:D + 1, 512:1024],
                                    v1_t[:, ki, 2 * hp + 1, :], p_t[:, 512:1024],
                                    start=(ki == 0), stop=(ki == nki - 1))
                            # softmax denominators: recip of PSUM row 64, bcast, mul
                            l_row = nrm.tile([1, 1024], F32, tag="l")
                            nc.vector.tensor_copy(l_row[:], o_ps[D:D + 1, :])
                            r_row = nrm.tile([1, 1024], F32, tag="r")
                            nc.vector.reciprocal_approx_fast(out=r_row[:], in_=l_row[:])
                            r_bc = nrm.tile([64, 1024], F32, tag="rb")
                            nc.gpsimd.partition_broadcast(r_bc[:], r_row[:], channels=64)
                            nc.vector.tensor_tensor(
                                on_t[0:64, hp, qr * 512:(qr + 1) * 512],
                                o_ps[0:64, 0:512], r_bc[:, 0:512],
                                mybir.AluOpType.mult)
                            ob_t = nrm.tile([64, 512], BF16, tag="ob")
                            nc.vector.tensor_tensor(
                                ob_t[:], o_ps[0:64, 512:1024], r_bc[:, 512:1024],
                                mybir.AluOpType.mult)
                            nc.sync.dma_start(
                                on_t[64:128, hp, qr * 512:(qr + 1) * 512], ob_t[:])

                # ---- phase 3: output projection ----
                with tc.tile_pool(name="py", bufs=2, space="PSUM") as py:
                    for tt in range(TT):
                        ya_ps = py.tile([128, 512], F32, tag="ya")
                        yb_ps = py.tile([128, 256], F32, tag="yb")
                        for hp in range(HP):
                            lhsT = on_t[:, hp, tt * 128:(tt + 1) * 128]
                            nc.tensor.matmul(ya_ps[:], lhsT, wp_t[:, hp, 0:512],
                                             start=(hp == 0), stop=(hp == HP - 1))
                            nc.tensor.matmul(yb_ps[:], lhsT, wp_t[:, hp, 512:768],
                                             start=(hp == 0), stop=(hp == HP - 1))
                        y_sb = work.tile([128, C], F32, tag="y")
                        nc.vector.tensor_copy(y_sb[:, 0:512], ya_ps[:])
                        nc.vector.tensor_copy(y_sb[:, 512:768], yb_ps[:])
                        nc.gpsimd.dma_start(y_d[tt * 128:(tt + 1) * 128, :], y_sb[:])

            if loop:
                loop_cm.__exit__(None, None, None)

    nc.finalize()
    return nc


def _causal_masks():
    f = np.arange(128)[None, :]
    k = np.arange(128)[:, None]
    return (f >= k).astype(BF)


def kernel(x, Wq, bq, Wk, bk, Wv, bv, Wp, bp):
    x, Wq, bq, Wk, bk, Wv, bv, Wp, bp = (
        np.asarray(a, dtype=np.float32)
        for a in (x, Wq, bq, Wk, bk, Wv, bv, Wp, bp))

    if "nc" not in _nc_cache:
        _nc_cache["nc"] = _build_nc()
    nc = _nc_cache["nc"]

    masks = _causal_masks()
    in_maps = []
    for c in range(NCORES):
        b, g = c // 2, c % 2
        sl = slice(g * CG, (g + 1) * CG)
        bqk = np.concatenate([bq[sl].reshape(HP, 128).T,
                              bk[sl].reshape(HP, 128).T], axis=1)
        in_maps.append({
            "xt": np.ascontiguousarray(x[b].T).astype(BF),
            "wq": np.ascontiguousarray(Wq[:, sl]).astype(BF),
            "wk": np.ascontiguousarray(Wk[:, sl]).astype(BF),
            "wv": np.ascontiguousarray(Wv[:, sl]).astype(BF),
            "wp": np.ascontiguousarray(Wp[sl, :]).astype(BF),
            "masks": masks,
            "bqk": np.ascontiguousarray(bqk).astype(np.float32),
            "bv_bc": np.tile(bv[sl][None, :], (128, 1)).astype(np.float32),
        })

    res = run_bass_kernel_spmd(nc, in_maps, core_ids=list(range(NCORES)))
    out = np.empty((B, T, C), np.float32)
    for b in range(B):
        out[b] = res.results[2 * b]["y"] + res.results[2 * b + 1]["y"] + bp
    return out

